# revision 22
# baseline (speedup 1.0000x reference)
"""VQ codebook squared-distance kernel for Trainium2 (8 NeuronCores).

Computes dist[n,k,l] = (||x[n,:,l]||^2 + ||w[k,:]||^2 - 2*x[n,:,l].w[k,:]) / scale^2
for x (32,128,3136) f32, weight (64,128) f32, scale (1,) f32 -> out (32,64,3136) f32.

Sharding: data-parallel over N (4 per core); weight/scale replicated.

v3 design notes (per-core; v1 37.5us, v2 38.4us measured):
  - Input stream is HBM-stack-roofline-bound (~343 GB/s/core with both
    NCs of a stack active): 6.42 MB f32 x read in ~17.4us. Not
    improvable; everything else is about the edges of the stream.
  - v1's killer: one SDMA engine (the doc'd engine-7/15 SWDGE
    descriptor-ring port contention) lags the other 15, and the lag
    grows whenever the Q7 ring is being written. The last input
    transfer's completion sem fired 3.8us after its bytes landed,
    gating the whole tail. Mitigation here: coarse transfers (full
    images for n0..n2 -> 128 descriptors each instead of 256+) and
    NO output descriptors on the SWDGE ring at all.
  - Outputs ride HWDGE (nc.sync) instead: no Q7 ring writes, no
    queueing behind the straggler's input backlog, faster dispatch
    (RTL descgen ~0.6us vs Q7 ~1.65us). Output pieces are gated late
    (pair-0 ships as one full-L write when its last epilogue lands
    ~24us) to limit read/write turnaround mixing during the stream.
  - The last image (n3) streams in shrinking pieces (4/2/1/1 chunks)
    so the final dependency chain after the last input byte is just:
    2 matmuls + one 392-col epilogue (split ACT||DVE halves) + one
    50 KB HWDGE write + ~2us completion receipt.
  - Output is offset fp8: e4m3(dist - 2D/s^2), host adds the offset
    back; rel_l2 ~3e-3 vs the 2e-2 budget, half the write traffic.
  - PE: psum = (-2Wt)f16 @ x_f16 + ones_f16 @ (x^2)_f16, two images
    per PSUM tile via column tiling (tile_position (0,0)/(0,64)).
  - NEFF postamble: walrus emits a FIXED 257 per-semaphore clears
    split across the 5 engines (measured invariant to kernel size);
    the pacer is the Tensor sequencer at ~115ns/clear when the HAM
    clock gate has re-throttled (PE idle >3.4us). A dummy matmul
    gated on the final epilogue keeps PE at K=8/8 through most of
    the clear window (~57ns/clear), moving the pacer to Scalar.
  - scale broadcast 1->128 via 1-col fp32 matmul; weight transpose on
    PE (identity built early on gpsimd).
"""

import numpy as np

N, D, L, K = 32, 128, 3136, 64
N_CORES = 8
NS = N // N_CORES          # n's per core
LC = 392                   # matmul chunk (8 per image, one PSUM bank)
HC = 196                   # half-chunk for the split tail epilogues

_cache = {}


def _build():
    import concourse.bacc as bacc
    import concourse.mybir as mybir
    import concourse.tile as tile
    from concourse.masks import make_identity

    f32 = mybir.dt.float32
    f16 = mybir.dt.float16
    f8 = mybir.dt.float8e4
    AF = mybir.ActivationFunctionType
    ALU = mybir.AluOpType

    nc = bacc.Bacc(
        "TRN2",
        target_bir_lowering=False,
        debug=False,
        enable_asserts=False,
        num_devices=N_CORES,
    )

    x_ap = nc.dram_tensor("x", (NS, D, L), f32, kind="ExternalInput").ap()
    w_ap = nc.dram_tensor("weight", (K, D), f32, kind="ExternalInput").ap()
    s_ap = nc.dram_tensor("scale", (1,), f32, kind="ExternalInput").ap()
    o_ap = nc.dram_tensor("out", (NS, K, L), f8, kind="ExternalOutput").ap()

    def ch(a, b):  # cols covering chunks [a, b)
        return slice(a * LC, b * LC)

    # Q0 (SWDGE cast-on-load) transfer plan: full images for the first
    # pair (their compute has slack), fine-grained interleaved pieces
    # for the second pair so each completion sem gates at most two
    # chunks of matmuls. Chunks 6-7 of n2/n3 arrive via HWDGE instead
    # (raw f32, cast on ACT) so the late tail has no SWDGE straggler
    # lag on its dependency sems.
    # 8 Q0 transfers total: the measured straggler lag was ~0 at 8
    # transfers (v3) and 1.6-4us at 10-14 (v1/v2/v4) — keep exactly 8.
    # Small head pieces so PE starts by ~11us; n2/n3 chunks 5-7 arrive
    # via HWDGE (raw f32) instead of Q0.
    stream = [
        (0, ch(0, 2)), (1, ch(0, 2)),
        (0, ch(2, 8)), (1, ch(2, 8)),
        (2, ch(0, 3)), (3, ch(0, 3)),
        (2, ch(3, 5)), (3, ch(3, 5)),
    ]

    with tile.TileContext(nc) as tc:
        with (
            tc.tile_pool(name="consts", bufs=1) as consts,
            tc.tile_pool(name="xin", bufs=4) as xpool,
            tc.tile_pool(name="xsq", bufs=4) as xqpool,
            tc.tile_pool(name="outp", bufs=2) as opool,
            tc.tile_pool(name="psum", bufs=4, space="PSUM") as pspool,
            tc.tile_pool(name="psum1", bufs=1, space="PSUM") as pspool1,
        ):
            xts = [
                xpool.tile([D, L], f16, tag="xt", name=f"x_{n}")
                for n in range(NS)
            ]
            xqs = [
                xqpool.tile([D, L], f16, tag="xq", name=f"xsq_{n}")
                for n in range(NS)
            ]

            # ---- input stream (SWDGE Q0, cast f32->f16 on load) ----------
            ident = consts.tile([K, K], f32)
            for i, (n, sl) in enumerate(stream):
                nc.gpsimd.dma_start(out=xts[n][:, sl], in_=x_ap[n][:, sl])
                if i == 0:
                    make_identity(nc, ident)

            # ---- HWDGE raw-f32 loads for chunks 5-7 of n2/n3 -------------
            # These drain alongside the Q0 stream and land mid-stream with
            # negligible completion lag. n2's cast to f16 runs on ACT,
            # n3's on DVE (keeps ACT's epilogue backlog smaller); both are
            # squared straight from f32, so chunks 5-7 of the last pair
            # are compute-ready well before the Q0 stream ends.
            xfs = {}
            for n in (2, 3):
                xf = xpool.tile([D, 3 * LC], f32, tag="xf", name=f"xf_{n}")
                xfs[n] = xf
                nc.sync.dma_start(out=xf, in_=x_ap[n][:, ch(5, 8)])
            nc.scalar.activation(xts[2][:, ch(5, 8)], xfs[2], AF.Identity)

            # ---- weight / scale prep (HWDGE, overlaps the stream) --------
            s_t = consts.tile([1, 1], f32)
            nc.sync.dma_start(out=s_t, in_=s_ap.to_broadcast((1, 1)))
            w2 = consts.tile([2 * K, D], f32)
            nc.sync.dma_start(out=w2[0:K, :], in_=w_ap)
            nc.sync.dma_start(out=w2[K : 2 * K, :], in_=w_ap)

            ones_row = consts.tile([1, 128], f32)
            nc.vector.memset(ones_row, 1.0)
            ones16 = consts.tile([D, K], f16)
            nc.vector.memset(ones16, 1.0)

            # broadcast scale to all 128 partitions via 1-col fp32 matmul
            ps_s = pspool1.tile([128, 1], f32, name="ps_s")
            nc.tensor.matmul(ps_s, ones_row, s_t, start=True, stop=True)
            s_b = consts.tile([128, 1], f32)
            nc.vector.tensor_scalar_mul(s_b, in0=ps_s, scalar1=1.0)
            inv_s2 = consts.tile([128, 1], f32)
            nc.vector.tensor_mul(inv_s2, s_b, s_b)
            nc.vector.reciprocal(inv_s2, inv_s2)

            w_sq = consts.tile([2 * K, D], f32)
            nc.vector.tensor_mul(w_sq, w2, w2)
            c_sq = consts.tile([2 * K, 1], f32)
            nc.vector.reduce_sum(out=c_sq, in_=w_sq, axis=mybir.AxisListType.X)
            c_sq_s = consts.tile([2 * K, 1], f32)
            nc.vector.tensor_mul(c_sq_s, c_sq, inv_s2)
            # fp8 offset encoding: store e4m3(dist - 2D/s^2); the host adds
            # the offset back. Centering kills the common mode so e4m3's
            # 6% relative step lands on the +-170 residual.
            bias2 = consts.tile([2 * K, 1], f32)
            nc.vector.tensor_scalar(
                out=bias2, in0=inv_s2,
                scalar1=-float(2 * D), scalar2=c_sq_s,
                op0=ALU.mult, op1=ALU.add,
            )

            ps_w = pspool1.tile([D, K], f32, name="ps_w")
            nc.tensor.transpose(ps_w, w2[0:K, :], ident)
            wT16 = consts.tile([D, K], f16)
            nc.vector.tensor_scalar_mul(wT16, in0=ps_w, scalar1=-2.0)

            # ---- derived stream: fp16 x^2 on DVE, in arrival order -------
            # (n3's HWDGE cast + both xf squares sit between the n0/n1
            # squares and the Q0 n2/n3 pieces, matching data arrival)
            for n, sl in stream[:4]:
                nc.vector.tensor_mul(xqs[n][:, sl], xts[n][:, sl], xts[n][:, sl])
            nc.vector.tensor_scalar_mul(
                xts[3][:, ch(5, 8)], in0=xfs[3], scalar1=1.0
            )
            for n in (2, 3):
                nc.vector.tensor_mul(xqs[n][:, ch(5, 8)], xfs[n], xfs[n])
            for n, sl in stream[4:]:
                nc.vector.tensor_mul(xqs[n][:, sl], xts[n][:, sl], xts[n][:, sl])

            # ---- matmuls + epilogues + HWDGE output pieces ---------------
            for pair in range(NS // 2):
                n0, n1 = 2 * pair, 2 * pair + 1
                out_t = opool.tile([2 * K, L], f8, tag="out_t", name=f"out_{pair}")
                o_pair = o_ap[2 * pair : 2 * pair + 2].rearrange("a k l -> (a k) l")
                last_pair = pair == NS // 2 - 1
                # pair 1's chunks are emitted in data-arrival order: the
                # HWDGE-fed chunks 5-7 are ready mid-stream, well before
                # the Q0-gated chunks; PE executes its queue in order.
                chunk_order = [5, 6, 7, 0, 1, 2, 3, 4] if last_pair else range(8)
                for c in chunk_order:
                    sl = ch(c, c + 1)
                    ps = pspool.tile([2 * K, LC], f32, name="ps")
                    nc.tensor.matmul(
                        ps[0:K, :], wT16, xts[n0][:, sl],
                        start=True, stop=False, tile_position=(0, 0),
                    )
                    nc.tensor.matmul(
                        ps[K : 2 * K, :], wT16, xts[n1][:, sl],
                        start=True, stop=False, tile_position=(0, 64),
                    )
                    nc.tensor.matmul(
                        ps[0:K, :], ones16, xqs[n0][:, sl],
                        start=False, stop=True, tile_position=(0, 0),
                    )
                    nc.tensor.matmul(
                        ps[K : 2 * K, :], ones16, xqs[n1][:, sl],
                        start=False, stop=True, tile_position=(0, 64),
                    )
                    if last_pair and c in (3, 4):
                        # split the late epilogues ACT || DVE so each clears
                        # in ~0.35us instead of ~0.7us (and ACT's backlog
                        # doesn't stack onto the final chunk)
                        nc.scalar.activation(
                            out_t[:, c * LC : c * LC + HC],
                            ps[:, 0:HC], AF.Identity,
                            bias=bias2, scale=inv_s2,
                        )
                        nc.vector.tensor_scalar(
                            out=out_t[:, c * LC + HC : (c + 1) * LC],
                            in0=ps[:, HC:LC],
                            scalar1=inv_s2, scalar2=bias2,
                            op0=ALU.mult, op1=ALU.add,
                        )
                    else:
                        nc.scalar.activation(
                            out_t[:, sl], ps, AF.Identity,
                            bias=bias2, scale=inv_s2,
                        )
                    # ship finished columns on HWDGE: pair 0 as one full-L
                    # write; pair 1 in pieces ordered by readiness, so the
                    # last-ready piece is the final 50 KB chunk ch(4,5).
                    if not last_pair:
                        if c == 7:
                            nc.sync.dma_start(out=o_pair, in_=out_t)
                    elif c == 7:
                        es = ch(5, 8)
                        nc.sync.dma_start(out=o_pair[:, es], in_=out_t[:, es])
                    elif c == 2:
                        hs = ch(0, 3)
                        nc.sync.dma_start(out=o_pair[:, hs], in_=out_t[:, hs])
                    elif c == 3:
                        qs = ch(3, 4)
                        nc.sync.dma_start(out=o_pair[:, qs], in_=out_t[:, qs])
                    elif c == 4:
                        fs = ch(4, 5)
                        nc.sync.dma_start(out=o_pair[:, fs], in_=out_t[:, fs])



    nc.compile()
    return nc


def _get_nc():
    if "nc" not in _cache:
        _cache["nc"] = _build()
    return _cache["nc"]


def run(x, weight, scale, trace=False, tmpdir=None):
    from concourse.bass_utils import run_bass_kernel_spmd

    x = np.ascontiguousarray(np.asarray(x, dtype=np.float32))
    weight = np.ascontiguousarray(np.asarray(weight, dtype=np.float32))
    scale = np.ascontiguousarray(np.asarray(scale, dtype=np.float32))
    assert x.shape == (N, D, L) and weight.shape == (K, D) and scale.shape == (1,)

    nc = _get_nc()
    in_maps = [
        {"x": x[c * NS : (c + 1) * NS], "weight": weight, "scale": scale}
        for c in range(N_CORES)
    ]
    res = run_bass_kernel_spmd(
        nc, in_maps, core_ids=list(range(N_CORES)), trace=trace, tmpdir=tmpdir
    )
    out = np.concatenate([r["out"] for r in res.results], axis=0).astype(np.float32)
    out += np.float32(2.0 * D) / np.float32(scale[0] ** 2)
    return out, res


def kernel(x, weight, scale):
    out, _ = run(x, weight, scale, trace=False)
    return out
